# revision 31
# baseline (speedup 1.0000x reference)
"""Trainium2 Bass kernel for nn_DigitCap (sparse_attention).

Math note: the reference's softmax is over a size-1 axis, so C == 1 exactly
and the whole N x N attention matrix A is dead code.  The computation
collapses to

    S[b,d,i]  = sum_{n,j} (1 + B[d,n]) * W[d,n,i,j] * U[b,n,j]
    out[b,d,:] = (1 - exp(-|S|)) * S / (|S| + 1e-7)

Sharding: split by digit capsule d (2 of 10 per core, zero-padded to a
uniform 2 so the SPMD program is identical on all 8 cores).

Perf structure (vs the fp32 baseline):
  * inputs stream as bf16 (tolerance is 2e-2; bf16 matmul with fp32 PSUM
    accumulate lands ~2.5e-3) -- halves DMA bytes and runs the PE at
    1 cycle/row instead of fp32's 4.
  * every HWDGE dma_start costs ~0.6us of sequencer issue time plus
    ~1.5us of gen/delay/sem-prop latency, so the 7 input DMAs are spread
    across all three HWDGE rings (SP, ACT, DVE) and U arrives in 4
    quarters so the PE starts on quarter 0 while the tail streams.
  * single PSUM bank accumulates all 32 chunks; epilogue squares +
    row-sums directly out of PSUM on the scalar engine (Square+accum),
    takes sqrt via exp(0.5*ln(x)) so Ln/Exp/Square all live in one ACT
    table set (warmed during the load phase -- no switch on the critical
    path), and overlaps the reciprocal on the DVE under the final Exp.
"""

import numpy as np
from contextlib import ExitStack

import ml_dtypes

import concourse.bass as bass
import concourse.mybir as mybir
from concourse.bass_utils import run_bass_kernel_spmd

F32 = mybir.dt.float32
BF16 = mybir.dt.bfloat16
NPBF16 = ml_dtypes.bfloat16
AF = mybir.ActivationFunctionType
ALU = mybir.AluOpType
P = 128
D, DD, N, DP = 10, 16, 512, 8     # digit caps, digit dim, primary caps, primary dim
K = N * DP                         # 4096 contraction
NCHUNK = K // P                    # 32 chunks of 128 contraction rows
NCORES = 8
BFULL = 64
DC = 2                             # d's per core (8*2 = 16 slots >= 10 real)
DIC = DC * DD                      # 32 output cols per core
HC = NCHUNK // 2                   # 16 chunks per half
QC = NCHUNK // 4                   # 8 chunks per U quarter
EPS = 1e-7


def build_raw():
    nc = bass.Bass()
    u_t = nc.dram_tensor("u_t", [P, NCHUNK * BFULL], BF16, kind="ExternalInput")
    w_t = nc.dram_tensor("w_t", [P, NCHUNK * DIC], BF16, kind="ExternalInput")
    bp = nc.dram_tensor("bp", [P, NCHUNK * DC], BF16, kind="ExternalInput")
    out = nc.dram_tensor("out", [BFULL, DIC], F32, kind="ExternalOutput")

    UH = HC * BFULL                # u cols per half (1024)
    WH = HC * DIC                  # w cols per half (512)

    with ExitStack() as ctx:
        u_all = ctx.enter_context(nc.sbuf_tensor("u_all", [P, NCHUNK * BFULL], BF16))
        w_all = ctx.enter_context(nc.sbuf_tensor("w_all", [P, NCHUNK * DIC], BF16))
        bsc = ctx.enter_context(nc.sbuf_tensor("bsc", [P, NCHUNK * DC], BF16))
        ps = ctx.enter_context(nc.psum_tensor("ps", [BFULL, DIC], F32))
        sq = ctx.enter_context(nc.sbuf_tensor("sq", [BFULL, DIC], F32))
        ss = ctx.enter_context(nc.sbuf_tensor("ss", [BFULL, DC], F32))
        lt = ctx.enter_context(nc.sbuf_tensor("lt", [BFULL, DC], F32))
        normt = ctx.enter_context(nc.sbuf_tensor("norm", [BFULL, DC], F32))
        den = ctx.enter_context(nc.sbuf_tensor("den", [BFULL, DC], F32))
        rec = ctx.enter_context(nc.sbuf_tensor("rec", [BFULL, DC], F32))
        et = ctx.enter_context(nc.sbuf_tensor("et", [BFULL, DC], F32))
        fac = ctx.enter_context(nc.sbuf_tensor("fac", [BFULL, DC], F32))
        ot = ctx.enter_context(nc.sbuf_tensor("ot", [BFULL, DIC], F32))
        warm = ctx.enter_context(nc.sbuf_tensor("warm", [1, 4], F32))
        s_w = ctx.enter_context(nc.semaphore("s_w"))
        s_u = [ctx.enter_context(nc.semaphore(f"s_u{h}")) for h in range(2)]
        s_bc = ctx.enter_context(nc.semaphore("s_bc"))
        s_wm = ctx.enter_context(nc.semaphore("s_wm"))
        s_dve = ctx.enter_context(nc.semaphore("s_dve"))
        s_pe = ctx.enter_context(nc.semaphore("s_pe"))
        s_a = ctx.enter_context(nc.semaphore("s_a"))
        s_nr = ctx.enter_context(nc.semaphore("s_nr"))
        s_e = ctx.enter_context(nc.semaphore("s_e"))
        s_v = ctx.enter_context(nc.semaphore("s_v"))
        s_fin = ctx.enter_context(nc.semaphore("s_fin"))
        s_out = ctx.enter_context(nc.semaphore("s_out"))

        with nc.Block() as block:

            @block.sync
            def _(sync):
                # W first: it gates the scale -> PE start.  DMA completion
                # latency (~3us issue->usable) dwarfs transfer time, so few
                # big DMAs beat many small ones.
                sync.dma_start(w_all[:], w_t[:, :]).then_inc(s_w, 16)
                for h in range(2):
                    sync.dma_start(
                        u_all[:, h * UH:(h + 1) * UH], u_t[:, h * UH:(h + 1) * UH]
                    ).then_inc(s_u[h], 16)
                sync.wait_ge(s_fin, 1)
                sync.dma_start(out[:, :], ot[:]).then_inc(s_out, 16)

            @block.vector
            def _(vector):
                # seed for the ACT table warm-up
                vector.memset(warm[:], 1.0).then_inc(s_wm, 1)
                # fused (bsc + 1) * W in two halves so PE can start early
                vector.wait_ge(s_bc, 16)
                vector.wait_ge(s_w, 16)
                for h in range(2):
                    w_v = w_all[:, h * WH:(h + 1) * WH].rearrange(
                        "p (c t i) -> p c t i", t=DC, i=DD
                    )
                    vector.scalar_tensor_tensor(
                        out=w_v,
                        in0=bsc[:, h * HC * DC:(h + 1) * HC * DC]
                        .rearrange("p (c t) -> p c t", t=DC)
                        .broadcast_to([P, HC, DC, DD]),
                        scalar=1.0,
                        in1=w_v,
                        op0=ALU.add,
                        op1=ALU.mult,
                    ).then_inc(s_dve, 1)
                # epilogue tail.  den = -(norm + eps), rec = 1/den < 0, and
                # fac = (et - 1) * rec = (1 - et)/(norm + eps); the whole
                # reciprocal branch runs under the ACT et.
                vector.wait_ge(s_nr, 1)
                vector.tensor_scalar(
                    out=den[:], in0=normt[:], scalar1=-1.0, scalar2=-EPS,
                    op0=ALU.mult, op1=ALU.add,
                ).then_inc(s_v, 1)
                vector.wait_ge(s_v, 1)
                vector.reciprocal(out=rec[:], in_=den[:]).then_inc(s_v, 1)
                vector.wait_ge(s_e, 1)
                vector.wait_ge(s_v, 2)
                vector.scalar_tensor_tensor(
                    out=fac[:], in0=et[:], scalar=1.0, in1=rec[:],
                    op0=ALU.subtract, op1=ALU.mult,
                ).then_inc(s_v, 1)
                vector.wait_ge(s_v, 3)
                vector.tensor_mul(
                    out=ot[:].rearrange("b (t i) -> b t i", i=DD),
                    in0=ps[:].rearrange("b (t i) -> b t i", i=DD),
                    in1=fac[:].broadcast_to([BFULL, DC, DD]),
                ).then_inc(s_fin, 1)

            @block.tensor
            def _(tensor):
                for g in range(2):
                    tensor.wait_ge(s_dve, g + 1)
                    tensor.wait_ge(s_u[g], 16)
                    for k in range(HC):
                        c = g * HC + k
                        mm = tensor.matmul(
                            ps[:],
                            lhsT=u_all[:, c * BFULL:(c + 1) * BFULL],
                            rhs=w_all[:, c * DIC:(c + 1) * DIC],
                            start=(c == 0), stop=(c == NCHUNK - 1),
                            skip_group_check=True,
                        )
                mm.then_inc(s_pe, 1)

            @block.scalar
            def _(scalar):
                # bsc on the ACT HWDGE ring
                scalar.dma_start(bsc[:], bp[:, :]).then_inc(s_bc, 16)
                # warm the natural_log_exp table set (Ln/Exp/Square all live
                # there) during the load phase -- no switch on critical path
                scalar.wait_ge(s_wm, 1)
                scalar.activation(out=warm[:, 0:1], in_=warm[:, 1:2], func=AF.Ln)
                scalar.wait_ge(s_wm, 1)
                scalar.activation(out=warm[:, 2:3], in_=warm[:, 1:2], func=AF.Exp)
                # epilogue head: ss[b,t] = sum_i S^2 straight out of PSUM,
                # then norm = exp(0.5 ln ss) = sqrt(ss), et = exp(-norm)
                scalar.wait_ge(s_pe, 1)
                for t in range(DC):
                    scalar.activation(
                        out=sq[:, t * DD:(t + 1) * DD],
                        in_=ps[:, t * DD:(t + 1) * DD],
                        func=AF.Square,
                        accum_out=ss[:, t:t + 1],
                    ).then_inc(s_a, 1)
                scalar.wait_ge(s_a, 2)
                scalar.activation(out=lt[:], in_=ss[:], func=AF.Ln).then_inc(s_a, 1)
                scalar.wait_ge(s_a, 3)
                scalar.activation(
                    out=normt[:], in_=lt[:], func=AF.Exp, scale=0.5
                ).then_inc(s_nr, 1)
                scalar.wait_ge(s_nr, 1)
                scalar.activation(
                    out=et[:], in_=normt[:], func=AF.Exp, scale=-1.0
                ).then_inc(s_e, 1)

    return nc


_CACHE = {}


def _get_nc():
    if "nc" not in _CACHE:
        _CACHE["nc"] = build_raw()
    return _CACHE["nc"]


def prep_inputs(primary_caps, W, B):
    """Host-side layout prep + sharding (no arithmetic).

    Contraction row order: chunk c holds n in [c*16, (c+1)*16); within a
    chunk, partition p = j*16 + n_local.  Core c owns digit caps
    d in {2c, 2c+1} (zeros for the 6 pad slots on cores 5-7).
    """
    U = np.asarray(primary_caps, dtype=np.float32)
    Wf = np.asarray(W, dtype=np.float32)
    Bf = np.asarray(B, dtype=np.float32).reshape(D, N)

    # U^T replicated: [p, (c b)]
    Unj = np.transpose(U, (1, 2, 0))  # n j b
    Ut = np.ascontiguousarray(
        Unj.reshape(NCHUNK, 16, DP, BFULL)
        .transpose(0, 2, 1, 3)
        .reshape(NCHUNK, P, BFULL)
        .transpose(1, 0, 2)
        .reshape(P, NCHUNK * BFULL)
    ).astype(NPBF16)

    # per-core W slice [p, (c, t, i)] and B slice [p, (c, t)]
    Wnj = np.transpose(Wf, (1, 3, 0, 2))  # n j d i
    Wc = (
        Wnj.reshape(NCHUNK, 16, DP, D, DD)
        .transpose(0, 2, 1, 3, 4)          # c j n_l d i
        .reshape(NCHUNK, P, D, DD)
        .transpose(1, 0, 2, 3)             # p c d i
    )
    Bn = Bf.reshape(D, NCHUNK, 16)         # d c n_l
    in_maps = []
    for core in range(NCORES):
        wt = np.zeros((P, NCHUNK, DC, DD), dtype=np.float32)
        bpt = np.zeros((16, NCHUNK, DC), dtype=np.float32)
        for t in range(DC):
            d = 2 * core + t
            if d < D:
                wt[:, :, t, :] = Wc[:, :, d, :]
                bpt[:, :, t] = Bn[d].T      # [n_l, c] -> ...
        bpm = np.ascontiguousarray(
            np.broadcast_to(
                bpt.reshape(1, 16, NCHUNK * DC), (DP, 16, NCHUNK * DC)
            ).reshape(P, NCHUNK * DC)
        ).astype(NPBF16)
        in_maps.append(
            {
                "u_t": Ut,
                "w_t": np.ascontiguousarray(
                    wt.reshape(P, NCHUNK * DIC)
                ).astype(NPBF16),
                "bp": bpm,
            }
        )
    return in_maps


def kernel(primary_caps, W, B):
    nc = _get_nc()
    in_maps = prep_inputs(primary_caps, W, B)
    res = run_bass_kernel_spmd(nc, in_maps, core_ids=list(range(NCORES)))
    full = np.empty((BFULL, D, DD), dtype=np.float32)
    for core in range(NCORES):
        o = res.results[core]["out"].reshape(BFULL, DC, DD)
        for t in range(DC):
            d = 2 * core + t
            if d < D:
                full[:, d, :] = o[:, t, :]
    return full


# revision 33
# speedup vs baseline: 1.0757x; 1.0757x over previous
"""Trainium2 Bass kernel for nn_DigitCap (sparse_attention).

Math note: the reference's softmax is over a size-1 axis, so C == 1 exactly
and the whole N x N attention matrix A is dead code.  The computation
collapses to

    S[b,d,i]  = sum_{n,j} (1 + B[d,n]) * W[d,n,i,j] * U[b,n,j]
    out[b,d,:] = (1 - exp(-|S|)) * S / (|S| + 1e-7)

Sharding: split by digit capsule d (2 of 10 per core, zero-padded to a
uniform 2 so the SPMD program is identical on all 8 cores).

Perf structure (vs the fp32 baseline):
  * inputs stream as bf16 (tolerance is 2e-2; bf16 matmul with fp32 PSUM
    accumulate lands ~2.5e-3) -- halves DMA bytes and runs the PE at
    1 cycle/row instead of fp32's 4.
  * every HWDGE dma_start costs ~0.6us of sequencer issue time plus
    ~1.5us of gen/delay/sem-prop latency, so the 7 input DMAs are spread
    across all three HWDGE rings (SP, ACT, DVE) and U arrives in 4
    quarters so the PE starts on quarter 0 while the tail streams.
  * single PSUM bank accumulates all 32 chunks; epilogue squares +
    row-sums directly out of PSUM on the scalar engine (Square+accum),
    takes sqrt via exp(0.5*ln(x)) so Ln/Exp/Square all live in one ACT
    table set (warmed during the load phase -- no switch on the critical
    path), and overlaps the reciprocal on the DVE under the final Exp.
"""

import numpy as np
from contextlib import ExitStack

import ml_dtypes

import concourse.bass as bass
import concourse.mybir as mybir
from concourse.bass_utils import run_bass_kernel_spmd

F32 = mybir.dt.float32
BF16 = mybir.dt.bfloat16
NPBF16 = ml_dtypes.bfloat16
AF = mybir.ActivationFunctionType
ALU = mybir.AluOpType
P = 128
D, DD, N, DP = 10, 16, 512, 8     # digit caps, digit dim, primary caps, primary dim
K = N * DP                         # 4096 contraction
NCHUNK = K // P                    # 32 chunks of 128 contraction rows
NCORES = 8
BFULL = 64
DC = 2                             # d's per core (8*2 = 16 slots >= 10 real)
DIC = DC * DD                      # 32 output cols per core
HC = NCHUNK // 2                   # 16 chunks per half
QC = NCHUNK // 4                   # 8 chunks per U quarter
EPS = 1e-7


def build_raw():
    nc = bass.Bass()
    u_t = nc.dram_tensor("u_t", [P, NCHUNK * BFULL], BF16, kind="ExternalInput")
    w_t = nc.dram_tensor("w_t", [P, NCHUNK * DIC], BF16, kind="ExternalInput")
    bp = nc.dram_tensor("bp", [P, NCHUNK * DC], BF16, kind="ExternalInput")
    out = nc.dram_tensor("out", [BFULL, DIC], F32, kind="ExternalOutput")

    UH = HC * BFULL                # u cols per half (1024)
    WH = HC * DIC                  # w cols per half (512)

    with ExitStack() as ctx:
        u_all = ctx.enter_context(nc.sbuf_tensor("u_all", [P, NCHUNK * BFULL], BF16))
        w_all = ctx.enter_context(nc.sbuf_tensor("w_all", [P, NCHUNK * DIC], BF16))
        bsc = ctx.enter_context(nc.sbuf_tensor("bsc", [P, NCHUNK * DC], BF16))
        ps = ctx.enter_context(nc.psum_tensor("ps", [BFULL, DIC], F32))
        sq = ctx.enter_context(nc.sbuf_tensor("sq", [BFULL, DIC], F32))
        ss = ctx.enter_context(nc.sbuf_tensor("ss", [BFULL, DC], F32))
        lt = ctx.enter_context(nc.sbuf_tensor("lt", [BFULL, DC], F32))
        normt = ctx.enter_context(nc.sbuf_tensor("norm", [BFULL, DC], F32))
        rec = ctx.enter_context(nc.sbuf_tensor("rec", [BFULL, DC], F32))
        et = ctx.enter_context(nc.sbuf_tensor("et", [BFULL, DC], F32))
        fac = ctx.enter_context(nc.sbuf_tensor("fac", [BFULL, DC], F32))
        ot = ctx.enter_context(nc.sbuf_tensor("ot", [BFULL, DIC], F32))
        warm = ctx.enter_context(nc.sbuf_tensor("warm", [1, 4], F32))
        s_w = ctx.enter_context(nc.semaphore("s_w"))
        s_u = [ctx.enter_context(nc.semaphore(f"s_u{h}")) for h in range(2)]
        s_bc = ctx.enter_context(nc.semaphore("s_bc"))
        s_wm = ctx.enter_context(nc.semaphore("s_wm"))
        s_dve = ctx.enter_context(nc.semaphore("s_dve"))
        s_pe = ctx.enter_context(nc.semaphore("s_pe"))
        s_a = ctx.enter_context(nc.semaphore("s_a"))
        s_nr = ctx.enter_context(nc.semaphore("s_nr"))
        s_e = ctx.enter_context(nc.semaphore("s_e"))
        s_v = ctx.enter_context(nc.semaphore("s_v"))
        s_fin = ctx.enter_context(nc.semaphore("s_fin"))
        s_out = ctx.enter_context(nc.semaphore("s_out"))

        with nc.Block() as block:

            @block.sync
            def _(sync):
                # W first: it gates the scale -> PE start.  DMA completion
                # latency (~3us issue->usable) dwarfs transfer time, so few
                # big DMAs beat many small ones.
                sync.dma_start(w_all[:], w_t[:, :]).then_inc(s_w, 16)
                for h in range(2):
                    sync.dma_start(
                        u_all[:, h * UH:(h + 1) * UH], u_t[:, h * UH:(h + 1) * UH]
                    ).then_inc(s_u[h], 16)
                sync.wait_ge(s_fin, 1)
                sync.dma_start(out[:, :], ot[:]).then_inc(s_out, 16)

            @block.vector
            def _(vector):
                # seed for the ACT table warm-up
                vector.memset(warm[:], 1.0).then_inc(s_wm, 1)
                # bscn = -(1 + B): the matmul then accumulates -S, and since
                # the final product ps * fac uses fac = (et - 1)/norm (also
                # negated), the signs cancel -- this saves the negation /
                # eps-add steps on the critical epilogue path.
                vector.wait_ge(s_bc, 16)
                vector.tensor_scalar(
                    out=bsc[:], in0=bsc[:], scalar1=-1.0, scalar2=-1.0,
                    op0=ALU.mult, op1=ALU.add,
                ).then_inc(s_v, 1)
                # fused bscn * W in two halves so PE can start early
                vector.wait_ge(s_v, 1)
                vector.wait_ge(s_w, 16)
                for h in range(2):
                    w_v = w_all[:, h * WH:(h + 1) * WH].rearrange(
                        "p (c t i) -> p c t i", t=DC, i=DD
                    )
                    vector.tensor_mul(
                        out=w_v,
                        in0=bsc[:, h * HC * DC:(h + 1) * HC * DC]
                        .rearrange("p (c t) -> p c t", t=DC)
                        .broadcast_to([P, HC, DC, DD]),
                        in1=w_v,
                    ).then_inc(s_dve, 1)
                # epilogue tail: rec = 1/norm runs under the ACT et, then
                # fac = (et - 1) * rec, ot = ps * fac  (ps = -S, fac = -f)
                vector.wait_ge(s_nr, 1)
                vector.reciprocal(out=rec[:], in_=normt[:]).then_inc(s_v, 1)
                vector.wait_ge(s_e, 1)
                vector.wait_ge(s_v, 2)
                vector.scalar_tensor_tensor(
                    out=fac[:], in0=et[:], scalar=1.0, in1=rec[:],
                    op0=ALU.subtract, op1=ALU.mult,
                ).then_inc(s_v, 1)
                vector.wait_ge(s_v, 3)
                vector.tensor_mul(
                    out=ot[:].rearrange("b (t i) -> b t i", i=DD),
                    in0=ps[:].rearrange("b (t i) -> b t i", i=DD),
                    in1=fac[:].broadcast_to([BFULL, DC, DD]),
                ).then_inc(s_fin, 1)

            @block.tensor
            def _(tensor):
                for g in range(2):
                    tensor.wait_ge(s_dve, g + 1)
                    tensor.wait_ge(s_u[g], 16)
                    for k in range(HC):
                        c = g * HC + k
                        mm = tensor.matmul(
                            ps[:],
                            lhsT=u_all[:, c * BFULL:(c + 1) * BFULL],
                            rhs=w_all[:, c * DIC:(c + 1) * DIC],
                            start=(c == 0), stop=(c == NCHUNK - 1),
                            skip_group_check=True,
                        )
                mm.then_inc(s_pe, 1)

            @block.scalar
            def _(scalar):
                # bsc on the ACT HWDGE ring
                scalar.dma_start(bsc[:], bp[:, :]).then_inc(s_bc, 16)
                # warm the natural_log_exp table set (Ln/Exp/Square all live
                # there) during the load phase -- no switch on critical path
                scalar.wait_ge(s_wm, 1)
                scalar.activation(out=warm[:, 0:1], in_=warm[:, 1:2], func=AF.Ln)
                scalar.wait_ge(s_wm, 1)
                scalar.activation(out=warm[:, 2:3], in_=warm[:, 1:2], func=AF.Exp)
                # epilogue head: ss[b,t] = sum_i S^2 straight out of PSUM,
                # then norm = exp(0.5 ln ss) = sqrt(ss), et = exp(-norm)
                scalar.wait_ge(s_pe, 1)
                for t in range(DC):
                    scalar.activation(
                        out=sq[:, t * DD:(t + 1) * DD],
                        in_=ps[:, t * DD:(t + 1) * DD],
                        func=AF.Square,
                        accum_out=ss[:, t:t + 1],
                    ).then_inc(s_a, 1)
                scalar.wait_ge(s_a, 2)
                scalar.activation(out=lt[:], in_=ss[:], func=AF.Ln).then_inc(s_a, 1)
                scalar.wait_ge(s_a, 3)
                scalar.activation(
                    out=normt[:], in_=lt[:], func=AF.Exp, scale=0.5
                ).then_inc(s_nr, 1)
                scalar.wait_ge(s_nr, 1)
                scalar.activation(
                    out=et[:], in_=normt[:], func=AF.Exp, scale=-1.0
                ).then_inc(s_e, 1)

    return nc


_CACHE = {}


def _get_nc():
    if "nc" not in _CACHE:
        _CACHE["nc"] = build_raw()
    return _CACHE["nc"]


def prep_inputs(primary_caps, W, B):
    """Host-side layout prep + sharding (no arithmetic).

    Contraction row order: chunk c holds n in [c*16, (c+1)*16); within a
    chunk, partition p = j*16 + n_local.  Core c owns digit caps
    d in {2c, 2c+1} (zeros for the 6 pad slots on cores 5-7).
    """
    U = np.asarray(primary_caps, dtype=np.float32)
    Wf = np.asarray(W, dtype=np.float32)
    Bf = np.asarray(B, dtype=np.float32).reshape(D, N)

    # U^T replicated: [p, (c b)]
    Unj = np.transpose(U, (1, 2, 0))  # n j b
    Ut = np.ascontiguousarray(
        Unj.reshape(NCHUNK, 16, DP, BFULL)
        .transpose(0, 2, 1, 3)
        .reshape(NCHUNK, P, BFULL)
        .transpose(1, 0, 2)
        .reshape(P, NCHUNK * BFULL)
    ).astype(NPBF16)

    # per-core W slice [p, (c, t, i)] and B slice [p, (c, t)]
    Wnj = np.transpose(Wf, (1, 3, 0, 2))  # n j d i
    Wc = (
        Wnj.reshape(NCHUNK, 16, DP, D, DD)
        .transpose(0, 2, 1, 3, 4)          # c j n_l d i
        .reshape(NCHUNK, P, D, DD)
        .transpose(1, 0, 2, 3)             # p c d i
    )
    Bn = Bf.reshape(D, NCHUNK, 16)         # d c n_l
    in_maps = []
    for core in range(NCORES):
        wt = np.zeros((P, NCHUNK, DC, DD), dtype=np.float32)
        bpt = np.zeros((16, NCHUNK, DC), dtype=np.float32)
        for t in range(DC):
            d = 2 * core + t
            if d < D:
                wt[:, :, t, :] = Wc[:, :, d, :]
                bpt[:, :, t] = Bn[d].T      # [n_l, c] -> ...
        bpm = np.ascontiguousarray(
            np.broadcast_to(
                bpt.reshape(1, 16, NCHUNK * DC), (DP, 16, NCHUNK * DC)
            ).reshape(P, NCHUNK * DC)
        ).astype(NPBF16)
        in_maps.append(
            {
                "u_t": Ut,
                "w_t": np.ascontiguousarray(
                    wt.reshape(P, NCHUNK * DIC)
                ).astype(NPBF16),
                "bp": bpm,
            }
        )
    return in_maps


def kernel(primary_caps, W, B):
    nc = _get_nc()
    in_maps = prep_inputs(primary_caps, W, B)
    res = run_bass_kernel_spmd(nc, in_maps, core_ids=list(range(NCORES)))
    full = np.empty((BFULL, D, DD), dtype=np.float32)
    for core in range(NCORES):
        o = res.results[core]["out"].reshape(BFULL, DC, DD)
        for t in range(DC):
            d = 2 * core + t
            if d < D:
                full[:, d, :] = o[:, t, :]
    return full
